# revision 3
# baseline (speedup 1.0000x reference)
"""Trainium2 Bass kernel for nn_BayesFittingNet (Gaussian NLL loss over 2M obs).

Math: loss = N*(0.5*32*log(2pi) + 0.5*logdet(P_post)) + 0.5 * sum_n quad_n
where quad_n = (obs_n - mu_post)^T Sigma_post (obs_n - mu_post).

sum_n quad_n = tr(Sigma_post @ G) - 2 mu^T Sigma_post s + N mu^T Sigma_post mu
with G = obs^T obs (16x16) and s = sum_n obs_n (16,). The device streams obs
once and produces per-core partial (G, s) via TensorE; the tiny 16-dim linear
algebra (and a 1152-row remainder) runs on the host in float64.

Device layout trick: a contiguous block of R rows (R % 128 == 0) maps to an
SBUF tile [128, R/8] (partition p holds R/128 consecutive rows). Any 128-wide
column slice Y_j of that tile holds 8 whole rows per partition, and the 16x16
diagonal blocks of Y_j^T @ Y_j are Gram matrices over disjoint row subsets.
Accumulating all Y_j^T Y_j into one PSUM [128,128] and summing its 8 diagonal
16x16 blocks on the host yields G exactly. s comes from Y_j^T @ ones.

Constraint notes: each DMA tile gets its own SBUF slot and the DMA count per
core is kept small — a rotating pool would attach two sync waits to each
DMACopy (the DIRECT2D pseudo-DMA has one wait slot), and the kernel-tail
Drain instruction also has a small wait budget (one wait per DMA lane used).
"""

import os
import sys
from contextlib import ExitStack

import numpy as np

for _p in ("/opt/trn_rl_repo", os.path.expanduser("~/.axon_site/_ro/trn_rl_repo")):
    if os.path.isdir(_p) and _p not in sys.path:
        sys.path.append(_p)

N_OBS = 2_000_000
DIM = 16
P = 128
N_CORES = 8
EPS = 1e-6
LOG_DIM = 32

R_MAIN = 249_856          # rows per core, = 1952 * 128
R_TAIL = N_OBS - N_CORES * R_MAIN   # 1152 rows, folded in on the host
# per-core DMA tile sizes in rows: small -> large for pipeline ramp-in,
# small at the end so the PE tail after the last DMA is negligible.
TILE_ROWS = tuple(1024 * u for u in
                  (2, 4, 8, 16, 32, 36, 36, 36, 32, 20, 8, 6, 4, 2, 1, 1))
assert sum(TILE_ROWS) == R_MAIN

LAST_RESULTS = None       # BassKernelResults of the most recent run (for test.py)
_BUILD_CACHE = {}


def build_bass(rows_main=R_MAIN, tile_rows=TILE_ROWS):
    """Raw-Bass builder (no TileContext): explicit per-engine programs and
    semaphores. The Tile layer is avoided on purpose — its end-of-kernel
    Drain packs one sync-wait per DMA lane into a single instruction, which
    this toolchain's walrus rejects ("Too many sync wait commands"); raw
    blocks emit each wait as its own instruction and also skip the ~10us
    end-of-kernel all-engine barrier butterfly.

    Engine split:
      gpsimd: SWDGE cast-DMAs (fp32 HBM -> bf16 SBUF), one per tile;
              8 semaphores reused with cumulative thresholds.
      tensor: per 128-column slice Y_j of each tile, accumulate
              Y_j^T Y_j into psum[:, :128] and Y_j^T ones into psum[:, 128]
              (same PSUM bank, disjoint columns).
      scalar: after the matmuls, one copy PSUM -> SBUF out tile and the
              final HWDGE DMA of the [128,129] out tile to DRAM.
    """
    import concourse.bass as bass
    from concourse import mybir

    assert sum(tile_rows) == rows_main
    assert all(r % P == 0 for r in tile_rows)
    f_total = rows_main * DIM // P

    nc = bass.Bass()
    obs_in = nc.dram_tensor("obs", [rows_main, DIM], mybir.dt.float32, kind="ExternalInput")
    out_ext = nc.dram_tensor("out", [P, 129], mybir.dt.float32, kind="ExternalOutput")

    # (fp32 elements per partition, f-offset in the slab buffer) per DMA tile
    specs = []
    f0 = 0
    for rows in tile_rows:
        f = rows * DIM // P
        specs.append((f, f0))
        f0 += f
    assert f0 == f_total
    n_mm = sum((f + P - 1) // P for f, _ in specs)

    # All tiles go over the SWDGE cast-DMA path. (Splitting a "head" onto the
    # HWDGE queue was tried and reverted: both queues share the 16 SDMA
    # engines round-robin at packet granularity, so the small HWDGE transfers
    # get starved by the SWDGE firehose and arrive ~30us late, stalling the
    # in-order PE stream far more than the ~0.5us earlier start saves.)
    with ExitStack() as ctx:
        bf_all = ctx.enter_context(
            nc.sbuf_tensor("bf_all", [P, f_total], mybir.dt.float8e4))
        ones_q8 = ctx.enter_context(
            nc.sbuf_tensor("ones_q8", [P, 1], mybir.dt.float8e4))
        out_sb = ctx.enter_context(
            nc.sbuf_tensor("out_sb", [P, 129], mybir.dt.float32))
        warm_sb = ctx.enter_context(
            nc.sbuf_tensor("warm_sb", [P, 1], mybir.dt.float32))
        psum_GS = ctx.enter_context(
            nc.psum_tensor("psum_GS", [P, P + 1], mybir.dt.float32))

        N_SW_SEMS = 8
        sw_sems = [ctx.enter_context(nc.semaphore(f"dma{t}"))
                   for t in range(min(N_SW_SEMS, len(specs)))]
        mm_sem = ctx.enter_context(nc.semaphore("mm_sem"))
        copy_sem = ctx.enter_context(nc.semaphore("copy_sem"))
        out_sem = ctx.enter_context(nc.semaphore("out_sem"))

        ones_f32 = nc.const_aps.aps[(mybir.dt.float32, 1.0)]
        # fp8 ones for the s-column matmul; the memset precedes the first
        # SWDGE emission on the same engine, so it lands before any matmul
        # can observe it (tensor waits on the first DMA's semaphore).
        nc.gpsimd.memset(ones_q8[:], 1.0)
        ones_bf = ones_q8[:]

        row_starts = []
        r0 = 0
        for rows in tile_rows:
            row_starts.append(r0)
            r0 += rows

        def src_ap(t):
            return obs_in[row_starts[t]:row_starts[t] + tile_rows[t], :].rearrange(
                "(p f) d -> p (f d)", p=P)

        # The first two DMAs are emitted in the entry basic block, BEFORE the
        # Block: the branch into each engine's body pays a ~0.9us IRAM fetch,
        # which now overlaps with these transfers instead of delaying them.
        N_PRE = min(2, len(specs))
        for t in range(N_PRE):
            f, f0_ = specs[t]
            nc.gpsimd.dma_start(out=bf_all[:, f0_:f0_ + f], in_=src_ap(t)
                                ).then_inc(sw_sems[t % N_SW_SEMS], 16)

        block = ctx.enter_context(nc.Block(no_gpsimd_drain=True))

        @block.gpsimd
        def _(gp: bass.BassEngine):
            for t in range(N_PRE, len(specs)):
                f, f0_ = specs[t]
                gp.dma_start(out=bf_all[:, f0_:f0_ + f], in_=src_ap(t)
                             ).then_inc(sw_sems[t % N_SW_SEMS], 16)

        @block.scalar
        def _(sc: bass.BassEngine):
            # Scalar both copies PSUM->SBUF and issues the output DMA (it has
            # a PSUM read port and is HWDGE-capable) — no inter-engine
            # semaphore hop between the copy and the DMA. No completion wait:
            # the Block-exit drains plus the multi-microsecond NEFF epilogue
            # run after the 66KB write is in flight.
            # Dummy 1-element copy first: ACT's first activation pays a
            # ~1.3us function-table load; do it here, during the stream,
            # instead of on the critical tail.
            sc.copy(warm_sb[:], ones_f32)
            sc.wait_ge(mm_sem, 1)
            sc.copy(out_sb[:], psum_GS[:]).then_inc(copy_sem, 1)
            sc.wait_ge(copy_sem, 1)  # ACT datapath must drain before DGE reads
            sc.dma_start(out=out_ext[:], in_=out_sb[:]).then_inc(out_sem, 16)

        @block.tensor
        def _(te: bass.BassEngine):
            mm = 0
            for t, (f, f0_) in enumerate(specs):
                te.wait_ge(sw_sems[t % N_SW_SEMS], 16 * (t // N_SW_SEMS + 1))
                for j0 in range(0, f, P):
                    w = min(P, f - j0)
                    lhsT = bf_all[:, f0_ + j0:f0_ + j0 + w]
                    # start=True only on the very first matmul: it clears the
                    # bank's has_written bits; every later write to a fresh
                    # element (including the s column) starts its own
                    # accumulation via the per-element has_written bit.
                    first = mm == 0
                    last = mm == n_mm - 1
                    te.matmul(psum_GS[0:w, 0:P][:, 0:w], lhsT, lhsT,
                              start=first, stop=last, skip_group_check=True)
                    mg = te.matmul(psum_GS[0:w, P:P + 1], lhsT, ones_bf,
                                   start=False, stop=last, skip_group_check=True)
                    if last:
                        mg.then_inc(mm_sem, 1)
                    mm += 1

    return nc


def _reduce_outputs(results):
    """Sum the 8 diagonal 16x16 blocks of each core's [128,129] output."""
    G = np.zeros((DIM, DIM), np.float64)
    s = np.zeros(DIM, np.float64)
    for r in results:
        o = np.asarray(r["out"], dtype=np.float64)
        for b in range(8):
            blk = slice(b * DIM, (b + 1) * DIM)
            G += o[blk, blk]
            s += o[blk, P]
    return G, s


def _block_diag_cov64(params):
    B = params.reshape(8, 2, 2)
    blocks = np.einsum("nij,nkj->nik", B, B) + EPS * np.eye(2)
    M = np.zeros((8, 2, 8, 2))
    for i in range(8):
        M[i, :, i, :] = blocks[i]
    return M.reshape(DIM, DIM)


def _finalize(G, s, mu_likelihood, mu_prior_pose, Sigma_prior_params, Sigma_likelihood_params):
    mu_l = np.asarray(mu_likelihood, np.float64)
    pose = np.asarray(mu_prior_pose, np.float64)
    Sp = _block_diag_cov64(np.asarray(Sigma_prior_params, np.float64))
    Sl = _block_diag_cov64(np.asarray(Sigma_likelihood_params, np.float64))

    Pp = np.linalg.inv(Sp)
    Pl = np.linalg.inv(Sl)
    Ppost = Pp + Pl
    S = np.linalg.inv(Ppost)
    L = np.linalg.cholesky(Ppost)
    logdet = 2.0 * np.sum(np.log(np.diag(L)))

    pts = np.stack([mu_l[0::2], mu_l[1::2]])
    c = pts.mean(axis=1, keepdims=True)
    ct, st = np.cos(pose[2]), np.sin(pose[2])
    R = np.array([[ct, -st], [st, ct]])
    pts = R @ (pts - c) + pose[:2, None]
    mu_prior = np.zeros(DIM)
    mu_prior[0::2] = pts[0]
    mu_prior[1::2] = pts[1]
    mu_post = S @ (Pp @ mu_prior + Pl @ mu_l)

    quad_sum = np.trace(S @ G) - 2.0 * mu_post @ S @ s + N_OBS * mu_post @ S @ mu_post
    loss = N_OBS * (0.5 * LOG_DIM * np.log(2.0 * np.pi) + 0.5 * logdet) + 0.5 * quad_sum
    return np.asarray(loss, dtype=np.float32)


def _ensure_axon_hooks():
    """bass_utils imports antenv.axon_hooks when BASS_TRACE is set under axon;
    some images lack that module. Provide a no-op fallback (hook=None makes
    bass_utils skip tracing gracefully) so a stray BASS_TRACE can't crash us."""
    try:
        import antenv.axon_hooks  # noqa: F401
    except ImportError:
        import types

        mod = types.ModuleType("antenv.axon_hooks")
        mod.get_axon_ntff_profile_hook = lambda: None
        mod.set_axon_ntff_profile_hook = lambda h: None
        sys.modules["antenv.axon_hooks"] = mod


def kernel(obs, mu_likelihood, mu_prior_pose, Sigma_prior_params, Sigma_likelihood_params):
    global LAST_RESULTS
    _ensure_axon_hooks()
    from concourse.bass_utils import run_bass_kernel_spmd

    obs = np.ascontiguousarray(np.asarray(obs, dtype=np.float32))
    assert obs.shape == (N_OBS, DIM)

    key = (R_MAIN, TILE_ROWS)
    nc = _BUILD_CACHE.get(key)
    if nc is None:
        nc = build_bass()
        _BUILD_CACHE[key] = nc

    in_maps = [{"obs": obs[c * R_MAIN:(c + 1) * R_MAIN]} for c in range(N_CORES)]
    res = run_bass_kernel_spmd(nc, in_maps, list(range(N_CORES)))
    LAST_RESULTS = res

    G, s = _reduce_outputs(res.results)

    # remainder rows, folded in exactly on the host
    tail = obs[N_CORES * R_MAIN:].astype(np.float64)
    G += tail.T @ tail
    s += tail.sum(axis=0)

    return _finalize(G, s, mu_likelihood, mu_prior_pose,
                     Sigma_prior_params, Sigma_likelihood_params)



# revision 4
# speedup vs baseline: 1.0578x; 1.0578x over previous
"""Trainium2 Bass kernel for nn_BayesFittingNet (Gaussian NLL loss over 2M obs).

Math: loss = N*(0.5*32*log(2pi) + 0.5*logdet(P_post)) + 0.5 * sum_n quad_n
where quad_n = (obs_n - mu_post)^T Sigma_post (obs_n - mu_post).

sum_n quad_n = tr(Sigma_post @ G) - 2 mu^T Sigma_post s + N mu^T Sigma_post mu
with G = obs^T obs (16x16) and s = sum_n obs_n (16,). The device streams obs
once and produces per-core partial G via TensorE; s and the tiny 16-dim
linear algebra run on the host in float64 (s is one exact pass over obs).

Device layout trick: a contiguous block of R rows (R % 128 == 0) maps to an
SBUF tile [128, R/8] (partition p holds R/128 consecutive rows). Any 128-wide
column slice Y_j of that tile holds 8 whole rows per partition, and the 16x16
diagonal blocks of Y_j^T @ Y_j are Gram matrices over disjoint row subsets.
Accumulating all Y_j^T Y_j into one PSUM [128,128] and summing its 8 diagonal
16x16 blocks on the host yields G exactly.

Perf notes (from perfetto traces):
  - The 16 SDMA engines stream the input gap-free at ~414 GB/s read-side;
    the kernel time is DMA-stream-bound plus a PE tail. Writing fp8 instead
    of bf16 to SBUF does NOT speed the stream (read/descriptor bound), and
    fp8 matmuls are not faster without DoubleRow, so bf16 is used.
  - Interleaving a per-slice [128,1] ones-matmul (for s) doubled the PE
    instruction count and broke LDWEIGHTS overlap: pairs ran at ~133ns
    mid-stream vs ~81ns in drain mode, leaving a ~13us PE backlog after the
    last DMA byte. Pure-Gram matmuls let PE track the stream; s moved to
    the host.
  - Big pre-Block tiles feed all 16 SDMA engines from ~2us; the in-Block
    emissions (Q7 starts them only at ~6us after the engine-boot sync)
    take over once the pre-Block data is draining.

Constraint notes: each DMA tile gets its own SBUF slot and the DMA count per
core is kept small -- a rotating pool would attach two sync waits to each
DMACopy (the DIRECT2D pseudo-DMA has one wait slot), and the kernel-tail
Drain instruction also has a small wait budget (one wait per DMA lane used).
"""

import os
import sys
from contextlib import ExitStack

import numpy as np

for _p in ("/opt/trn_rl_repo", os.path.expanduser("~/.axon_site/_ro/trn_rl_repo")):
    if os.path.isdir(_p) and _p not in sys.path:
        sys.path.append(_p)

N_OBS = 2_000_000
DIM = 16
P = 128
N_CORES = 8
EPS = 1e-6
LOG_DIM = 32

R_MAIN = 249_856          # rows per core, = 1952 * 128
R_TAIL = N_OBS - N_CORES * R_MAIN   # 1152 rows, folded in on the host
# per-core DMA tile sizes in rows: two big pre-Block tiles saturate the SDMA
# engines from ~2us; ramp-out keeps the PE tail after the last DMA negligible.
TILE_ROWS = tuple(1024 * u for u in
                  (16, 24, 32, 40, 40, 36, 28, 12, 8, 4, 2, 1, 1))
assert sum(TILE_ROWS) == R_MAIN

LAST_RESULTS = None       # BassKernelResults of the most recent run (for test.py)
_BUILD_CACHE = {}


def build_bass(rows_main=R_MAIN, tile_rows=TILE_ROWS):
    """Raw-Bass builder (no TileContext): explicit per-engine programs and
    semaphores. The Tile layer is avoided on purpose -- its end-of-kernel
    Drain packs one sync-wait per DMA lane into a single instruction, which
    this toolchain's walrus rejects ("Too many sync wait commands"); raw
    blocks emit each wait as its own instruction and also skip the ~10us
    end-of-kernel all-engine barrier butterfly.

    Engine split:
      gpsimd: SWDGE cast-DMAs (fp32 HBM -> bf16 SBUF), one per tile;
              8 semaphores reused with cumulative thresholds.
      tensor: per 128-column slice Y_j of each tile, accumulate
              Y_j^T Y_j into psum [128,128].
      scalar: after the matmuls, one copy PSUM -> SBUF out tile and the
              final HWDGE DMA of the [128,128] out tile to DRAM.
    """
    import concourse.bass as bass
    from concourse import mybir

    assert sum(tile_rows) == rows_main
    assert all(r % P == 0 for r in tile_rows)
    f_total = rows_main * DIM // P

    nc = bass.Bass()
    obs_in = nc.dram_tensor("obs", [rows_main, DIM], mybir.dt.float32, kind="ExternalInput")
    out_ext = nc.dram_tensor("out", [P, P], mybir.dt.float32, kind="ExternalOutput")

    # (fp32 elements per partition, f-offset in the slab buffer) per DMA tile
    specs = []
    f0 = 0
    for rows in tile_rows:
        f = rows * DIM // P
        specs.append((f, f0))
        f0 += f
    assert f0 == f_total
    n_mm = sum((f + P - 1) // P for f, _ in specs)

    # All tiles go over the SWDGE cast-DMA path. (Splitting a "head" onto the
    # HWDGE queue was tried and reverted: both queues share the 16 SDMA
    # engines round-robin at packet granularity, so the small HWDGE transfers
    # get starved by the SWDGE firehose and arrive ~30us late, stalling the
    # in-order PE stream far more than the ~0.5us earlier start saves.)
    with ExitStack() as ctx:
        bf_all = ctx.enter_context(
            nc.sbuf_tensor("bf_all", [P, f_total], mybir.dt.bfloat16))
        out_sb = ctx.enter_context(
            nc.sbuf_tensor("out_sb", [P, P], mybir.dt.float32))
        warm_sb = ctx.enter_context(
            nc.sbuf_tensor("warm_sb", [P, 1], mybir.dt.float32))
        psum_G = ctx.enter_context(
            nc.psum_tensor("psum_G", [P, P], mybir.dt.float32))

        N_SW_SEMS = 8
        sw_sems = [ctx.enter_context(nc.semaphore(f"dma{t}"))
                   for t in range(min(N_SW_SEMS, len(specs)))]
        mm_sem = ctx.enter_context(nc.semaphore("mm_sem"))
        copy_sem = ctx.enter_context(nc.semaphore("copy_sem"))
        out_sem = ctx.enter_context(nc.semaphore("out_sem"))

        ones_f32 = nc.const_aps.aps[(mybir.dt.float32, 1.0)]

        row_starts = []
        r0 = 0
        for rows in tile_rows:
            row_starts.append(r0)
            r0 += rows

        def src_ap(t):
            return obs_in[row_starts[t]:row_starts[t] + tile_rows[t], :].rearrange(
                "(p f) d -> p (f d)", p=P)

        # The first two DMAs are emitted in the entry basic block, BEFORE the
        # Block: they start flowing at ~2us, well before the ~5.5us engine
        # boot sync that gates the in-Block gpsimd body. They are sized to
        # keep all 16 SDMA engines fed until the in-Block emissions (first
        # data ~8.7us) take over.
        N_PRE = min(2, len(specs))
        for t in range(N_PRE):
            f, f0_ = specs[t]
            nc.gpsimd.dma_start(out=bf_all[:, f0_:f0_ + f], in_=src_ap(t)
                                ).then_inc(sw_sems[t % N_SW_SEMS], 16)

        block = ctx.enter_context(nc.Block(no_gpsimd_drain=True))

        @block.gpsimd
        def _(gp: bass.BassEngine):
            for t in range(N_PRE, len(specs)):
                f, f0_ = specs[t]
                gp.dma_start(out=bf_all[:, f0_:f0_ + f], in_=src_ap(t)
                             ).then_inc(sw_sems[t % N_SW_SEMS], 16)

        @block.scalar
        def _(sc: bass.BassEngine):
            # Scalar both copies PSUM->SBUF and issues the output DMA (it has
            # a PSUM read port and is HWDGE-capable) -- no inter-engine
            # semaphore hop between the copy and the DMA. No completion wait:
            # the Block-exit drains plus the multi-microsecond NEFF epilogue
            # run after the 64KB write is in flight.
            # Dummy 1-element copy first: ACT's first activation pays a
            # ~1.3us function-table load; do it here, during the stream,
            # instead of on the critical tail.
            sc.copy(warm_sb[:], ones_f32)
            sc.wait_ge(mm_sem, 1)
            sc.copy(out_sb[:], psum_G[:]).then_inc(copy_sem, 1)
            sc.wait_ge(copy_sem, 1)  # ACT datapath must drain before DGE reads
            sc.dma_start(out=out_ext[:], in_=out_sb[:]).then_inc(out_sem, 16)

        @block.tensor
        def _(te: bass.BassEngine):
            mm = 0
            for t, (f, f0_) in enumerate(specs):
                te.wait_ge(sw_sems[t % N_SW_SEMS], 16 * (t // N_SW_SEMS + 1))
                for j0 in range(0, f, P):
                    w = min(P, f - j0)
                    lhsT = bf_all[:, f0_ + j0:f0_ + j0 + w]
                    # start=True only on the very first matmul: it clears the
                    # bank's has_written bits; every later write to a fresh
                    # element starts its own accumulation via the per-element
                    # has_written bit.
                    first = mm == 0
                    last = mm == n_mm - 1
                    mg = te.matmul(psum_G[0:w, 0:P][:, 0:w], lhsT, lhsT,
                                   start=first, stop=last, skip_group_check=True)
                    if last:
                        mg.then_inc(mm_sem, 1)
                    mm += 1

    return nc


def _reduce_outputs(results):
    """Sum the 8 diagonal 16x16 blocks of each core's [128,128] output."""
    G = np.zeros((DIM, DIM), np.float64)
    for r in results:
        o = np.asarray(r["out"], dtype=np.float64)
        for b in range(8):
            blk = slice(b * DIM, (b + 1) * DIM)
            G += o[blk, blk]
    return G


def _block_diag_cov64(params):
    B = params.reshape(8, 2, 2)
    blocks = np.einsum("nij,nkj->nik", B, B) + EPS * np.eye(2)
    M = np.zeros((8, 2, 8, 2))
    for i in range(8):
        M[i, :, i, :] = blocks[i]
    return M.reshape(DIM, DIM)


def _finalize(G, s, mu_likelihood, mu_prior_pose, Sigma_prior_params, Sigma_likelihood_params):
    mu_l = np.asarray(mu_likelihood, np.float64)
    pose = np.asarray(mu_prior_pose, np.float64)
    Sp = _block_diag_cov64(np.asarray(Sigma_prior_params, np.float64))
    Sl = _block_diag_cov64(np.asarray(Sigma_likelihood_params, np.float64))

    Pp = np.linalg.inv(Sp)
    Pl = np.linalg.inv(Sl)
    Ppost = Pp + Pl
    S = np.linalg.inv(Ppost)
    L = np.linalg.cholesky(Ppost)
    logdet = 2.0 * np.sum(np.log(np.diag(L)))

    pts = np.stack([mu_l[0::2], mu_l[1::2]])
    c = pts.mean(axis=1, keepdims=True)
    ct, st = np.cos(pose[2]), np.sin(pose[2])
    R = np.array([[ct, -st], [st, ct]])
    pts = R @ (pts - c) + pose[:2, None]
    mu_prior = np.zeros(DIM)
    mu_prior[0::2] = pts[0]
    mu_prior[1::2] = pts[1]
    mu_post = S @ (Pp @ mu_prior + Pl @ mu_l)

    quad_sum = np.trace(S @ G) - 2.0 * mu_post @ S @ s + N_OBS * mu_post @ S @ mu_post
    loss = N_OBS * (0.5 * LOG_DIM * np.log(2.0 * np.pi) + 0.5 * logdet) + 0.5 * quad_sum
    return np.asarray(loss, dtype=np.float32)


def _ensure_axon_hooks():
    """bass_utils imports antenv.axon_hooks when BASS_TRACE is set under axon;
    some images lack that module. Provide a no-op fallback (hook=None makes
    bass_utils skip tracing gracefully) so a stray BASS_TRACE can't crash us."""
    try:
        import antenv.axon_hooks  # noqa: F401
    except ImportError:
        import types

        mod = types.ModuleType("antenv.axon_hooks")
        mod.get_axon_ntff_profile_hook = lambda: None
        mod.set_axon_ntff_profile_hook = lambda h: None
        sys.modules["antenv.axon_hooks"] = mod


def kernel(obs, mu_likelihood, mu_prior_pose, Sigma_prior_params, Sigma_likelihood_params):
    global LAST_RESULTS
    _ensure_axon_hooks()
    from concourse.bass_utils import run_bass_kernel_spmd

    obs = np.ascontiguousarray(np.asarray(obs, dtype=np.float32))
    assert obs.shape == (N_OBS, DIM)

    key = (R_MAIN, TILE_ROWS)
    nc = _BUILD_CACHE.get(key)
    if nc is None:
        nc = build_bass()
        _BUILD_CACHE[key] = nc

    in_maps = [{"obs": obs[c * R_MAIN:(c + 1) * R_MAIN]} for c in range(N_CORES)]
    res = run_bass_kernel_spmd(nc, in_maps, list(range(N_CORES)))
    LAST_RESULTS = res

    G = _reduce_outputs(res.results)

    # remainder rows, folded in exactly on the host
    tail = obs[N_CORES * R_MAIN:].astype(np.float64)
    G += tail.T @ tail

    # s over ALL rows, exact, one host pass
    s = obs.sum(axis=0, dtype=np.float64)

    return _finalize(G, s, mu_likelihood, mu_prior_pose,
                     Sigma_prior_params, Sigma_likelihood_params)


# revision 6
# speedup vs baseline: 1.0610x; 1.0030x over previous
"""Trainium2 Bass kernel for nn_BayesFittingNet (Gaussian NLL loss over 2M obs).

Math: loss = N*(0.5*32*log(2pi) + 0.5*logdet(P_post)) + 0.5 * sum_n quad_n
where quad_n = (obs_n - mu_post)^T Sigma_post (obs_n - mu_post).

sum_n quad_n = tr(Sigma_post @ G) - 2 mu^T Sigma_post s + N mu^T Sigma_post mu
with G = obs^T obs (16x16) and s = sum_n obs_n (16,). The device streams obs
once and produces per-core partial G via TensorE; s and the tiny 16-dim
linear algebra run on the host in float64 (s is one exact pass over obs).

Device layout trick: a contiguous block of R rows (R % 128 == 0) maps to an
SBUF tile [128, R/8] (partition p holds R/128 consecutive rows). Any 128-wide
column slice Y_j of that tile holds 8 whole rows per partition, and the 16x16
diagonal blocks of Y_j^T @ Y_j are Gram matrices over disjoint row subsets.
Accumulating all Y_j^T Y_j into one PSUM [128,128] and summing its 8 diagonal
16x16 blocks on the host yields G exactly.

Perf notes (from perfetto traces):
  - The 16 SDMA engines stream the input gap-free at ~414 GB/s read-side;
    the kernel time is DMA-stream-bound plus a PE tail. Writing fp8 instead
    of bf16 to SBUF does NOT speed the stream (read/descriptor bound), and
    fp8 matmuls are not faster without DoubleRow, so bf16 is used.
  - Interleaving a per-slice [128,1] ones-matmul (for s) doubled the PE
    instruction count and broke LDWEIGHTS overlap: pairs ran at ~133ns
    mid-stream vs ~81ns in drain mode, leaving a ~13us PE backlog after the
    last DMA byte. Pure-Gram matmuls let PE track the stream; s moved to
    the host.
  - Big pre-Block tiles feed all 16 SDMA engines from ~2us; the in-Block
    emissions (Q7 starts them only at ~6us after the engine-boot sync)
    take over once the pre-Block data is draining.

Constraint notes: each DMA tile gets its own SBUF slot and the DMA count per
core is kept small -- a rotating pool would attach two sync waits to each
DMACopy (the DIRECT2D pseudo-DMA has one wait slot), and the kernel-tail
Drain instruction also has a small wait budget (one wait per DMA lane used).
"""

import os
import sys
from contextlib import ExitStack

import numpy as np

for _p in ("/opt/trn_rl_repo", os.path.expanduser("~/.axon_site/_ro/trn_rl_repo")):
    if os.path.isdir(_p) and _p not in sys.path:
        sys.path.append(_p)

N_OBS = 2_000_000
DIM = 16
P = 128
N_CORES = 8
EPS = 1e-6
LOG_DIM = 32

R_MAIN = 249_856          # rows per core, = 1952 * 128
R_TAIL = N_OBS - N_CORES * R_MAIN   # 1152 rows, folded in on the host
# per-core DMA tile sizes in rows: two big pre-Block tiles saturate the SDMA
# engines from ~2us; ramp-out keeps the PE tail after the last DMA negligible.
TILE_ROWS = tuple(1024 * u for u in
                  (16, 24, 40, 40, 40, 36, 28, 12, 4, 2, 2))
assert sum(TILE_ROWS) == R_MAIN

LAST_RESULTS = None       # BassKernelResults of the most recent run (for test.py)
_BUILD_CACHE = {}


def build_bass(rows_main=R_MAIN, tile_rows=TILE_ROWS):
    """Raw-Bass builder (no TileContext): explicit per-engine programs and
    semaphores. The Tile layer is avoided on purpose -- its end-of-kernel
    Drain packs one sync-wait per DMA lane into a single instruction, which
    this toolchain's walrus rejects ("Too many sync wait commands"); raw
    blocks emit each wait as its own instruction and also skip the ~10us
    end-of-kernel all-engine barrier butterfly.

    Engine split:
      gpsimd: SWDGE cast-DMAs (fp32 HBM -> bf16 SBUF), one per tile;
              8 semaphores reused with cumulative thresholds.
      tensor: per 128-column slice Y_j of each tile, accumulate
              Y_j^T Y_j into psum [128,128].
      scalar: after the matmuls, one copy PSUM -> SBUF out tile and the
              final HWDGE DMA of the [128,128] out tile to DRAM.
    """
    import concourse.bass as bass
    from concourse import mybir

    assert sum(tile_rows) == rows_main
    assert all(r % P == 0 for r in tile_rows)
    f_total = rows_main * DIM // P

    nc = bass.Bass()
    obs_in = nc.dram_tensor("obs", [rows_main, DIM], mybir.dt.float32, kind="ExternalInput")
    out_ext = nc.dram_tensor("out", [P, P], mybir.dt.float32, kind="ExternalOutput")

    # (fp32 elements per partition, f-offset in the slab buffer) per DMA tile
    specs = []
    f0 = 0
    for rows in tile_rows:
        f = rows * DIM // P
        specs.append((f, f0))
        f0 += f
    assert f0 == f_total
    n_mm = sum((f + P - 1) // P for f, _ in specs)

    # All tiles go over the SWDGE cast-DMA path. (Splitting a "head" onto the
    # HWDGE queue was tried and reverted: both queues share the 16 SDMA
    # engines round-robin at packet granularity, so the small HWDGE transfers
    # get starved by the SWDGE firehose and arrive ~30us late, stalling the
    # in-order PE stream far more than the ~0.5us earlier start saves.)
    with ExitStack() as ctx:
        bf_all = ctx.enter_context(
            nc.sbuf_tensor("bf_all", [P, f_total], mybir.dt.bfloat16))
        out_sb = ctx.enter_context(
            nc.sbuf_tensor("out_sb", [P, P], mybir.dt.float32))
        warm_sb = ctx.enter_context(
            nc.sbuf_tensor("warm_sb", [P, 1], mybir.dt.float32))
        psum_G = ctx.enter_context(
            nc.psum_tensor("psum_G", [P, P], mybir.dt.float32))

        N_SW_SEMS = 8
        sw_sems = [ctx.enter_context(nc.semaphore(f"dma{t}"))
                   for t in range(min(N_SW_SEMS, len(specs)))]
        mm_sem = ctx.enter_context(nc.semaphore("mm_sem"))
        copy_sem = ctx.enter_context(nc.semaphore("copy_sem"))
        out_sem = ctx.enter_context(nc.semaphore("out_sem"))

        ones_f32 = nc.const_aps.aps[(mybir.dt.float32, 1.0)]

        row_starts = []
        r0 = 0
        for rows in tile_rows:
            row_starts.append(r0)
            r0 += rows

        def src_ap(t):
            return obs_in[row_starts[t]:row_starts[t] + tile_rows[t], :].rearrange(
                "(p f) d -> p (f d)", p=P)

        # ALL DMAs are emitted in the entry basic block, BEFORE the Block:
        # Q7's in-Block body only starts at ~6us (engine boot sync), whereas
        # entry-block emissions start flowing at ~2us. Descriptors for every
        # tile are queued up front; SDMA engines join the drain as their
        # rings come live.
        for t in range(len(specs)):
            f, f0_ = specs[t]
            nc.gpsimd.dma_start(out=bf_all[:, f0_:f0_ + f], in_=src_ap(t)
                                ).then_inc(sw_sems[t % N_SW_SEMS], 16)

        block = ctx.enter_context(nc.Block(no_gpsimd_drain=True))

        @block.gpsimd
        def _(gp: bass.BassEngine):
            pass

        @block.scalar
        def _(sc: bass.BassEngine):
            # Scalar both copies PSUM->SBUF and issues the output DMA (it has
            # a PSUM read port and is HWDGE-capable) -- no inter-engine
            # semaphore hop between the copy and the DMA. No completion wait:
            # the Block-exit drains plus the multi-microsecond NEFF epilogue
            # run after the 64KB write is in flight.
            # Dummy 1-element copy first: ACT's first activation pays a
            # ~1.3us function-table load; do it here, during the stream,
            # instead of on the critical tail.
            sc.copy(warm_sb[:], ones_f32)
            sc.wait_ge(mm_sem, 1)
            sc.copy(out_sb[:], psum_G[:]).then_inc(copy_sem, 1)
            sc.wait_ge(copy_sem, 1)  # ACT datapath must drain before DGE reads
            sc.dma_start(out=out_ext[:], in_=out_sb[:]).then_inc(out_sem, 16)

        @block.tensor
        def _(te: bass.BassEngine):
            mm = 0
            for t, (f, f0_) in enumerate(specs):
                te.wait_ge(sw_sems[t % N_SW_SEMS], 16 * (t // N_SW_SEMS + 1))
                for j0 in range(0, f, P):
                    w = min(P, f - j0)
                    lhsT = bf_all[:, f0_ + j0:f0_ + j0 + w]
                    # start=True only on the very first matmul: it clears the
                    # bank's has_written bits; every later write to a fresh
                    # element starts its own accumulation via the per-element
                    # has_written bit.
                    first = mm == 0
                    last = mm == n_mm - 1
                    mg = te.matmul(psum_G[0:w, 0:P][:, 0:w], lhsT, lhsT,
                                   start=first, stop=last, skip_group_check=True)
                    if last:
                        mg.then_inc(mm_sem, 1)
                    mm += 1

    return nc


def _reduce_outputs(results):
    """Sum the 8 diagonal 16x16 blocks of each core's [128,128] output."""
    G = np.zeros((DIM, DIM), np.float64)
    for r in results:
        o = np.asarray(r["out"], dtype=np.float64)
        for b in range(8):
            blk = slice(b * DIM, (b + 1) * DIM)
            G += o[blk, blk]
    return G


def _block_diag_cov64(params):
    B = params.reshape(8, 2, 2)
    blocks = np.einsum("nij,nkj->nik", B, B) + EPS * np.eye(2)
    M = np.zeros((8, 2, 8, 2))
    for i in range(8):
        M[i, :, i, :] = blocks[i]
    return M.reshape(DIM, DIM)


def _finalize(G, s, mu_likelihood, mu_prior_pose, Sigma_prior_params, Sigma_likelihood_params):
    mu_l = np.asarray(mu_likelihood, np.float64)
    pose = np.asarray(mu_prior_pose, np.float64)
    Sp = _block_diag_cov64(np.asarray(Sigma_prior_params, np.float64))
    Sl = _block_diag_cov64(np.asarray(Sigma_likelihood_params, np.float64))

    Pp = np.linalg.inv(Sp)
    Pl = np.linalg.inv(Sl)
    Ppost = Pp + Pl
    S = np.linalg.inv(Ppost)
    L = np.linalg.cholesky(Ppost)
    logdet = 2.0 * np.sum(np.log(np.diag(L)))

    pts = np.stack([mu_l[0::2], mu_l[1::2]])
    c = pts.mean(axis=1, keepdims=True)
    ct, st = np.cos(pose[2]), np.sin(pose[2])
    R = np.array([[ct, -st], [st, ct]])
    pts = R @ (pts - c) + pose[:2, None]
    mu_prior = np.zeros(DIM)
    mu_prior[0::2] = pts[0]
    mu_prior[1::2] = pts[1]
    mu_post = S @ (Pp @ mu_prior + Pl @ mu_l)

    quad_sum = np.trace(S @ G) - 2.0 * mu_post @ S @ s + N_OBS * mu_post @ S @ mu_post
    loss = N_OBS * (0.5 * LOG_DIM * np.log(2.0 * np.pi) + 0.5 * logdet) + 0.5 * quad_sum
    return np.asarray(loss, dtype=np.float32)


def _ensure_axon_hooks():
    """bass_utils imports antenv.axon_hooks when BASS_TRACE is set under axon;
    some images lack that module. Provide a no-op fallback (hook=None makes
    bass_utils skip tracing gracefully) so a stray BASS_TRACE can't crash us."""
    try:
        import antenv.axon_hooks  # noqa: F401
    except ImportError:
        import types

        mod = types.ModuleType("antenv.axon_hooks")
        mod.get_axon_ntff_profile_hook = lambda: None
        mod.set_axon_ntff_profile_hook = lambda h: None
        sys.modules["antenv.axon_hooks"] = mod


def kernel(obs, mu_likelihood, mu_prior_pose, Sigma_prior_params, Sigma_likelihood_params):
    global LAST_RESULTS
    _ensure_axon_hooks()
    from concourse.bass_utils import run_bass_kernel_spmd

    obs = np.ascontiguousarray(np.asarray(obs, dtype=np.float32))
    assert obs.shape == (N_OBS, DIM)

    key = (R_MAIN, TILE_ROWS)
    nc = _BUILD_CACHE.get(key)
    if nc is None:
        nc = build_bass()
        _BUILD_CACHE[key] = nc

    in_maps = [{"obs": obs[c * R_MAIN:(c + 1) * R_MAIN]} for c in range(N_CORES)]
    res = run_bass_kernel_spmd(nc, in_maps, list(range(N_CORES)))
    LAST_RESULTS = res

    G = _reduce_outputs(res.results)

    # remainder rows, folded in exactly on the host
    tail = obs[N_CORES * R_MAIN:].astype(np.float64)
    G += tail.T @ tail

    # s over ALL rows, exact, one host pass
    s = obs.sum(axis=0, dtype=np.float64)

    return _finalize(G, s, mu_likelihood, mu_prior_pose,
                     Sigma_prior_params, Sigma_likelihood_params)


# revision 7
# speedup vs baseline: 1.1943x; 1.1255x over previous
"""Trainium2 Bass kernel for nn_BayesFittingNet (Gaussian NLL loss over 2M obs).

Math: loss = N*(0.5*32*log(2pi) + 0.5*logdet(P_post)) + 0.5 * sum_n quad_n
where quad_n = (obs_n - mu_post)^T Sigma_post (obs_n - mu_post).

sum_n quad_n = tr(Sigma_post @ G) - 2 mu^T Sigma_post s + N mu^T Sigma_post mu
with G = obs^T obs (16x16) and s = sum_n obs_n (16,). The device streams obs
once and produces per-core partial G via TensorE; s and the tiny 16-dim
linear algebra run on the host in float64 (s is one exact pass over obs).

Device layout trick: a contiguous block of R rows (R % 128 == 0) maps to an
SBUF tile [128, R/8] (partition p holds R/128 consecutive rows). Any 128-wide
column slice Y_j of that tile holds 8 whole rows per partition, and the 16x16
diagonal blocks of Y_j^T @ Y_j are Gram matrices over disjoint row subsets.
Accumulating all Y_j^T Y_j into one PSUM [128,128] and summing its 8 diagonal
16x16 blocks on the host yields G exactly.

Perf notes (from perfetto traces of earlier revisions):
  - SWDGE (gpsimd cast-DMA) streams leave ONE straggler SDMA engine ~11us
    behind the other 15 (descriptor-ring traffic on its SBUF port), and every
    tile's completion semaphore waits on it. HWDGE (sync/scalar) generates
    descriptors in RTL with no SBUF rings, so the input stream now goes over
    HWDGE as plain fp32 (HWDGE cannot cast).
  - PE reads the fp32 slab through a stride-2 bfloat16 view (the high half
    of each fp32 IS its truncated bf16) -- no conversion pass at all.
  - Nothing user-visible executes before the ~5.3us engine boot; entry-BB
    DMA emission starts at boot rather than after the ~1us Block entry.
  - A per-slice [128,1] ones-matmul for s doubled PE instruction count and
    broke LDWEIGHTS overlap (133 vs 81ns/pair); s moved to the host.
"""

import os
import sys
from contextlib import ExitStack

import numpy as np

for _p in ("/opt/trn_rl_repo", os.path.expanduser("~/.axon_site/_ro/trn_rl_repo")):
    if os.path.isdir(_p) and _p not in sys.path:
        sys.path.append(_p)

N_OBS = 2_000_000
DIM = 16
P = 128
N_CORES = 8
EPS = 1e-6
LOG_DIM = 32

R_MAIN = 249_856          # rows per core, = 1952 * 128
R_TAIL = N_OBS - N_CORES * R_MAIN   # 1152 rows, folded in on the host
# per-core DMA tile sizes in rows: small first tile for an early PE start,
# big middle tiles, small tail so the post-stream PE work is negligible.
TILE_ROWS = tuple(1024 * u for u in
                  (4, 8, 16, 32, 40, 40, 40, 32, 16, 8, 4, 2, 2))
assert sum(TILE_ROWS) == R_MAIN

LAST_RESULTS = None       # BassKernelResults of the most recent run (for test.py)
_BUILD_CACHE = {}


def build_bass(rows_main=R_MAIN, tile_rows=TILE_ROWS):
    """Raw-Bass builder (no TileContext): explicit per-engine programs and
    semaphores.

    Engine split:
      sync (SP): HWDGE input DMAs (fp32 HBM -> fp32 SBUF), one per tile,
              emitted in the entry basic block; 8 semaphores reused with
              cumulative thresholds.
      tensor: per 128-column bf16-view slice Y_j of each tile, accumulate
              Y_j^T Y_j into psum [128,128].
      scalar: after the matmuls, one copy PSUM -> SBUF out tile and the
              final HWDGE DMA of the [128,128] out tile to DRAM.
      gpsimd: idle (no SWDGE -> no descriptor-ring SBUF traffic).
    """
    import concourse.bass as bass
    from concourse import mybir

    assert sum(tile_rows) == rows_main
    assert all(r % P == 0 for r in tile_rows)
    f_total = rows_main * DIM // P

    nc = bass.Bass()
    obs_in = nc.dram_tensor("obs", [rows_main, DIM], mybir.dt.float32, kind="ExternalInput")
    out_ext = nc.dram_tensor("out", [P, P], mybir.dt.float32, kind="ExternalOutput")

    # (fp32 elements per partition, f-offset in the slab buffer) per DMA tile
    specs = []
    f0 = 0
    for rows in tile_rows:
        f = rows * DIM // P
        specs.append((f, f0))
        f0 += f
    assert f0 == f_total
    n_mm = sum((f + P - 1) // P for f, _ in specs)

    with ExitStack() as ctx:
        slab = ctx.enter_context(
            nc.sbuf_tensor("slab", [P, f_total], mybir.dt.float32))
        out_sb = ctx.enter_context(
            nc.sbuf_tensor("out_sb", [P, P], mybir.dt.float32))
        warm_sb = ctx.enter_context(
            nc.sbuf_tensor("warm_sb", [P, 1], mybir.dt.float32))
        psum_G = ctx.enter_context(
            nc.psum_tensor("psum_G", [P, P], mybir.dt.float32))

        N_SW_SEMS = 8
        sw_sems = [ctx.enter_context(nc.semaphore(f"dma{t}"))
                   for t in range(min(N_SW_SEMS, len(specs)))]
        mm_sem = ctx.enter_context(nc.semaphore("mm_sem"))
        copy_sem = ctx.enter_context(nc.semaphore("copy_sem"))
        out_sem = ctx.enter_context(nc.semaphore("out_sem"))

        ones_f32 = nc.const_aps.aps[(mybir.dt.float32, 1.0)]

        # bf16 view of the fp32 slab: element k's high half (bytes 4k+2..3)
        # is fp32 value k truncated to bf16 (little-endian).
        hi_view = slab[:, :].bitcast(mybir.dt.bfloat16).rearrange(
            "p (f two) -> p f two", two=2)[:, :, 1]

        row_starts = []
        r0 = 0
        for rows in tile_rows:
            row_starts.append(r0)
            r0 += rows

        def src_ap(t):
            return obs_in[row_starts[t]:row_starts[t] + tile_rows[t], :].rearrange(
                "(p f) d -> p (f d)", p=P)

        # All input DMAs emitted in SP's entry basic block: HWDGE descriptor
        # generation is RTL-side, the instructions just queue up and the
        # 16 SDMA engines drain the ring in FIFO order from ~boot+0.6us.
        for t in range(len(specs)):
            f, f0_ = specs[t]
            nc.sync.dma_start(out=slab[:, f0_:f0_ + f], in_=src_ap(t)
                              ).then_inc(sw_sems[t % N_SW_SEMS], 16)

        block = ctx.enter_context(nc.Block(no_gpsimd_drain=True))

        @block.gpsimd
        def _(gp: bass.BassEngine):
            pass

        @block.scalar
        def _(sc: bass.BassEngine):
            # Scalar both copies PSUM->SBUF and issues the output DMA (it has
            # a PSUM read port and is HWDGE-capable) -- no inter-engine
            # semaphore hop between the copy and the DMA. No completion wait:
            # the Block-exit drains plus the multi-microsecond NEFF epilogue
            # run after the 64KB write is in flight.
            # Dummy 1-element copy first: ACT's first activation pays a
            # ~1.3us function-table load; do it here, during the stream,
            # instead of on the critical tail.
            sc.copy(warm_sb[:], ones_f32)
            sc.wait_ge(mm_sem, 1)
            sc.copy(out_sb[:], psum_G[:]).then_inc(copy_sem, 1)
            sc.wait_ge(copy_sem, 1)  # ACT datapath must drain before DGE reads
            sc.dma_start(out=out_ext[:], in_=out_sb[:]).then_inc(out_sem, 16)

        @block.tensor
        def _(te: bass.BassEngine):
            mm = 0
            for t, (f, f0_) in enumerate(specs):
                te.wait_ge(sw_sems[t % N_SW_SEMS], 16 * (t // N_SW_SEMS + 1))
                for j0 in range(0, f, P):
                    w = min(P, f - j0)
                    lhsT = hi_view[:, f0_ + j0:f0_ + j0 + w]
                    # start=True only on the very first matmul: it clears the
                    # bank's has_written bits; every later write to a fresh
                    # element starts its own accumulation via the per-element
                    # has_written bit.
                    first = mm == 0
                    last = mm == n_mm - 1
                    mg = te.matmul(psum_G[0:w, 0:P][:, 0:w], lhsT, lhsT,
                                   start=first, stop=last, skip_group_check=True)
                    if last:
                        mg.then_inc(mm_sem, 1)
                    mm += 1

    return nc


def _reduce_outputs(results):
    """Sum the 8 diagonal 16x16 blocks of each core's [128,128] output."""
    G = np.zeros((DIM, DIM), np.float64)
    for r in results:
        o = np.asarray(r["out"], dtype=np.float64)
        for b in range(8):
            blk = slice(b * DIM, (b + 1) * DIM)
            G += o[blk, blk]
    return G


def _block_diag_cov64(params):
    B = params.reshape(8, 2, 2)
    blocks = np.einsum("nij,nkj->nik", B, B) + EPS * np.eye(2)
    M = np.zeros((8, 2, 8, 2))
    for i in range(8):
        M[i, :, i, :] = blocks[i]
    return M.reshape(DIM, DIM)


def _finalize(G, s, mu_likelihood, mu_prior_pose, Sigma_prior_params, Sigma_likelihood_params):
    mu_l = np.asarray(mu_likelihood, np.float64)
    pose = np.asarray(mu_prior_pose, np.float64)
    Sp = _block_diag_cov64(np.asarray(Sigma_prior_params, np.float64))
    Sl = _block_diag_cov64(np.asarray(Sigma_likelihood_params, np.float64))

    Pp = np.linalg.inv(Sp)
    Pl = np.linalg.inv(Sl)
    Ppost = Pp + Pl
    S = np.linalg.inv(Ppost)
    L = np.linalg.cholesky(Ppost)
    logdet = 2.0 * np.sum(np.log(np.diag(L)))

    pts = np.stack([mu_l[0::2], mu_l[1::2]])
    c = pts.mean(axis=1, keepdims=True)
    ct, st = np.cos(pose[2]), np.sin(pose[2])
    R = np.array([[ct, -st], [st, ct]])
    pts = R @ (pts - c) + pose[:2, None]
    mu_prior = np.zeros(DIM)
    mu_prior[0::2] = pts[0]
    mu_prior[1::2] = pts[1]
    mu_post = S @ (Pp @ mu_prior + Pl @ mu_l)

    quad_sum = np.trace(S @ G) - 2.0 * mu_post @ S @ s + N_OBS * mu_post @ S @ mu_post
    loss = N_OBS * (0.5 * LOG_DIM * np.log(2.0 * np.pi) + 0.5 * logdet) + 0.5 * quad_sum
    return np.asarray(loss, dtype=np.float32)


def _ensure_axon_hooks():
    """bass_utils imports antenv.axon_hooks when BASS_TRACE is set under axon;
    some images lack that module. Provide a no-op fallback (hook=None makes
    bass_utils skip tracing gracefully) so a stray BASS_TRACE can't crash us."""
    try:
        import antenv.axon_hooks  # noqa: F401
    except ImportError:
        import types

        mod = types.ModuleType("antenv.axon_hooks")
        mod.get_axon_ntff_profile_hook = lambda: None
        mod.set_axon_ntff_profile_hook = lambda h: None
        sys.modules["antenv.axon_hooks"] = mod


def kernel(obs, mu_likelihood, mu_prior_pose, Sigma_prior_params, Sigma_likelihood_params):
    global LAST_RESULTS
    _ensure_axon_hooks()
    from concourse.bass_utils import run_bass_kernel_spmd

    obs = np.ascontiguousarray(np.asarray(obs, dtype=np.float32))
    assert obs.shape == (N_OBS, DIM)

    key = (R_MAIN, TILE_ROWS)
    nc = _BUILD_CACHE.get(key)
    if nc is None:
        nc = build_bass()
        _BUILD_CACHE[key] = nc

    in_maps = [{"obs": obs[c * R_MAIN:(c + 1) * R_MAIN]} for c in range(N_CORES)]
    res = run_bass_kernel_spmd(nc, in_maps, list(range(N_CORES)))
    LAST_RESULTS = res

    G = _reduce_outputs(res.results)

    # remainder rows, folded in exactly on the host; the device saw bf16-
    # truncated values, the host tail uses float64 -- both well inside the
    # 2e-2 gate.
    tail = obs[N_CORES * R_MAIN:].astype(np.float64)
    G += tail.T @ tail

    # s over ALL rows, exact, one host pass
    s = obs.sum(axis=0, dtype=np.float64)

    return _finalize(G, s, mu_likelihood, mu_prior_pose,
                     Sigma_prior_params, Sigma_likelihood_params)
